# revision 31
# baseline (speedup 1.0000x reference)
"""Trainium2 Bass kernel for nn_Attention_v2_cross (dense transformer, 8 cores).

Sharding: 8 cores = 4 batches x 2 query-halves. Every core holds the full
weights and full context for its batch (kv projection duplicated across the
pair, zero collectives).

v3 design:
  - Everything SBUF-resident: q (fp8 e4m3) packed [(r,d), i] per head,
    k (fp8) packed [(r,d), j], v (bf16) packed [j, (r,d)]. Only the
    attention output is staged through DRAM (bf16) for the out-projection.
  - ALL matmuls in bf16/fp8 (fp32r lowers to 2-pass fp32_mode=H, 2x slower):
    x/ctx are converted fp32->bf16 on the fly during streaming, split
    between the scalar and vector engines.
  - sim is computed TRANSPOSED: simT[j, i] = k_chunk^T @ q, so exp output
    is directly the attn@v moving operand -- no PE transposes at all.
  - No row-max subtraction (softmax is shift-invariant, logits are O(1)).
  - P = exp(sim) accumulated unnormalized; each head's output scaled by
    1/den on the PSUM->SBUF copy (den = ones-matmul over PT; reciprocal
    broadcast across partitions by a K=1 matmul).
  - LN1 folded into the q projection: q = (W x + (-mean) (x) Wsum) * inv.
  - ctx streamed twice (two 4-head groups) so k/v fit in SBUF.
  - Final LN fused: Square(pf + bias) with accum_out gives the variance
    row-sum in one scalar instruction.
"""

import os
import numpy as np

B, N, R, C = 4, 1024, 12, 512
H, D = 8, 64
E = H * D            # 512
NQ = N // 2          # 512 queries per core
NKJ = N              # 1024 keys per core
ALPHA = 128.0
EPS = 1e-5
XCOLS = R * NQ       # 6144  (col = r*NQ + i)
CCOLS = R * NKJ      # 12288 (col = r*NKJ + j)
P = 128
KC = (R * D) // P    # 6 contraction chunks of 128 over (r,d)
JC = NKJ // P        # 8 key blocks of 128
SIM_SCALE = (D ** -0.5) * (R ** -0.5)   # exp scale; ALPHA cancels in softmax

_CACHE = {}


def _build_program():
    from contextlib import ExitStack
    import concourse.bass as bass
    import concourse.tile as tile
    from concourse import bacc
    from concourse import mybir

    F32 = mybir.dt.float32
    BF16 = mybir.dt.bfloat16
    FP8 = mybir.dt.float8e4
    AF = mybir.ActivationFunctionType
    AX = mybir.AxisListType.X

    nc = bacc.Bacc("TRN2", target_bir_lowering=False, debug=False, num_devices=8)

    xT = nc.dram_tensor("xT", [C, XCOLS], F32, kind="ExternalInput").ap()
    ctxT = nc.dram_tensor("ctxT", [C, CCOLS], F32, kind="ExternalInput").ap()
    wqT = nc.dram_tensor("wqT", [C, E], BF16, kind="ExternalInput").ap()
    wkT = nc.dram_tensor("wkT", [C, E], BF16, kind="ExternalInput").ap()
    wvT = nc.dram_tensor("wvT", [C, E], BF16, kind="ExternalInput").ap()
    wqsum = nc.dram_tensor("wqsum", [1, E], BF16, kind="ExternalInput").ap()
    woT = nc.dram_tensor("woT", [E, C], BF16, kind="ExternalInput").ap()
    nullk = nc.dram_tensor("nullk", [P, 1], F32, kind="ExternalInput").ap()
    nullv = nc.dram_tensor("nullv", [1, P], F32, kind="ExternalInput").ap()
    out = nc.dram_tensor("out", [XCOLS, C], F32, kind="ExternalOutput").ap()

    with ExitStack() as ctx:
        tc = ctx.enter_context(tile.TileContext(nc))

        const = ctx.enter_context(tc.tile_pool(name="const", bufs=1))
        dram = ctx.enter_context(tc.tile_pool(name="dram", bufs=1, space="DRAM"))

        ones_mat = const.tile([P, P], BF16)         # column-sum + broadcast
        nc.vector.memset(ones_mat[:], 1.0)
        ones_row = const.tile([1, P], BF16)         # K=1 partition broadcast
        nc.vector.memset(ones_row[:], 1.0)
        nullk_f = const.tile([P, 1], F32)
        nc.sync.dma_start(nullk_f[:], nullk[:, :])
        nullk_m = const.tile([P, P], FP8)           # null key replicated 128x
        nc.any.tensor_copy(nullk_m[:], nullk_f[:].to_broadcast((P, P)))
        nullv_f = const.tile([1, P], F32)
        nc.sync.dma_start(nullv_f[:], nullv[:, :])
        nullv_s = const.tile([1, P], BF16)
        nc.any.tensor_copy(nullv_s[:], nullv_f[:])
        wqsum_s = const.tile([1, E], BF16)
        nc.sync.dma_start(wqsum_s[:], wqsum[:, :])
        eps_1 = const.tile([1, 1], F32)
        nc.vector.memset(eps_1[:], EPS)
        eps_P = const.tile([P, 1], F32)
        nc.vector.memset(eps_P[:], EPS)

        aoT_d = dram.tile([P, 4, XCOLS], BF16)   # aoT[e, (r,i)]: e = ec*128+p

        with tc.tile_pool(name="res", bufs=1) as res, \
             tc.tile_pool(name="w", bufs=1) as wpool, \
             tc.tile_pool(name="sf", bufs=8) as sf, \
             tc.tile_pool(name="sb", bufs=6) as sb, \
             tc.tile_pool(name="sq", bufs=4) as sq_p, \
             tc.tile_pool(name="stat", bufs=2) as stat, \
             tc.tile_pool(name="pt", bufs=2) as pt_p, \
             tc.tile_pool(name="ao", bufs=3) as ao_p, \
             tc.tile_pool(name="ps", bufs=1, space="PSUM") as ps:

            # Resident activations
            q_res = res.tile([P, H, KC, NQ], FP8)          # [(r%2)*64+d, h, kchunk, i]
            k_res = res.tile([P, 4, KC, NKJ], FP8)         # per group: 4 heads
            v_res = res.tile([P, JC, 4, R * D], BF16)      # [j%128, jc, h, (r,d)]

            wq_s = wpool.tile([P, 4, E], BF16)
            wk_s = wpool.tile([P, 4, E], BF16)
            wv_s = wpool.tile([P, 4, E], BF16)

            # ------------- Stage A helpers (LN(x) folded into q proj) ----
            def dma_x(rb):
                tiles = []
                for cc in range(4):
                    t = sf.tile([P, NQ], F32, tag="xf", bufs=6)
                    nc.sync.dma_start(t[:], xT[cc * P : (cc + 1) * P, rb * NQ : (rb + 1) * NQ])
                    tiles.append(t)
                return tiles

            def conv_x(fts):
                tiles = []
                for cc in range(4):
                    xb = sb.tile([P, NQ], BF16, tag="xb", bufs=8)
                    if cc % 2 == 0:
                        nc.scalar.activation(xb[:], fts[cc][:], AF.Copy)
                    else:
                        nc.vector.tensor_copy(xb[:], fts[cc][:])
                    tiles.append(xb)
                return tiles

            def a_block(rb, xbs):
                psum_sum = ps.tile([P, NQ], F32, tag="big", bufs=6)
                for cc in range(4):
                    nc.tensor.matmul(psum_sum[:], ones_mat[:], xbs[cc][:],
                                     start=(cc == 0), stop=(cc == 3))
                # negm first: frees the stats PSUM buffer early
                negm_b = stat.tile([P, NQ], BF16, tag="negm")
                nc.scalar.mul(negm_b[:], psum_sum[:], -1.0 / C)
                kc_half, poff = rb // 2, (rb % 2) * D
                pqs = []
                for ec in range(4):
                    pq = ps.tile([P, NQ], F32, tag="big", bufs=6)
                    for cc in range(4):
                        nc.tensor.matmul(
                            pq[:], wq_s[:, cc, ec * P : (ec + 1) * P],
                            xbs[cc][:], start=(cc == 0), stop=False)
                    pqs.append(pq)
                sqs = []
                for cc in range(4):
                    sq = sq_p.tile([P, NQ], BF16, tag="sq")
                    if cc % 2 == 0:
                        nc.vector.tensor_mul(sq[:], xbs[cc][:], xbs[cc][:])
                    else:
                        nc.scalar.activation(sq[:], xbs[cc][:], AF.Square)
                    sqs.append(sq)
                psum_sq = ps.tile([P, NQ], F32, tag="big", bufs=6)
                for cc in range(4):
                    nc.tensor.matmul(psum_sq[:], ones_mat[:], sqs[cc][:],
                                     start=(cc == 0), stop=(cc == 3))
                for ec in range(4):
                    nc.tensor.matmul(
                        pqs[ec][:], wqsum_s[:, ec * P : (ec + 1) * P],
                        negm_b[0:1, :], start=False, stop=True)
                msq = stat.tile([P, NQ], F32, tag="msq")
                nc.vector.tensor_mul(msq[:], negm_b[:], negm_b[:])
                var = stat.tile([P, NQ], F32, tag="var")
                nc.vector.scalar_tensor_tensor(
                    var[:], psum_sq[:], 1.0 / C, msq[:],
                    mybir.AluOpType.mult, mybir.AluOpType.subtract)
                lnv = stat.tile([P, NQ], F32, tag="var")
                nc.scalar.activation(lnv[:], var[:], AF.Ln, bias=eps_P[:])
                a_s = stat.tile([P, NQ], F32, tag="a_s")
                nc.scalar.activation(a_s[:], lnv[:], AF.Exp, scale=-0.5)
                for ec in range(4):
                    for hh in range(2):
                        h = 2 * ec + hh
                        dst = q_res[poff : poff + D, h, kc_half, :]
                        src = pqs[ec][hh * D : (hh + 1) * D, :]
                        av = a_s[hh * D : (hh + 1) * D, :]
                        nc.vector.tensor_mul(dst, src, av)

            for cc in range(4):
                nc.sync.dma_start(wk_s[:, cc, :], wkT[cc * P : (cc + 1) * P, :])
                nc.sync.dma_start(wv_s[:, cc, :], wvT[cc * P : (cc + 1) * P, :])

            def dma_ctx(cb):
                tiles = []
                for cc in range(4):
                    t = sf.tile([P, NQ], F32, tag="cf")
                    nc.sync.dma_start(
                        t[:], ctxT[cc * P : (cc + 1) * P, cb * NQ : (cb + 1) * NQ])
                    tiles.append(t)
                return tiles

            def conv_ctx(fts):
                tiles = []
                for cc in range(4):
                    cbt = sb.tile([P, NQ], BF16, tag="cb", bufs=10)
                    if cc % 2 == 0:
                        nc.scalar.activation(cbt[:], fts[cc][:], AF.Copy)
                    else:
                        nc.vector.tensor_copy(cbt[:], fts[cc][:])
                    tiles.append(cbt)
                return tiles

            def b_block(g, cb, cbs):
                e0 = g * 256
                r, jh = cb // 2, cb % 2
                kc_half, poff = r // 2, (r % 2) * D
                # k projection (2 e-chunks = 4 heads), transposed out [e, j]
                for e2 in range(2):
                    ec = 2 * g + e2
                    pk = ps.tile([P, NQ], F32, tag="big", bufs=6)
                    for cc in range(4):
                        nc.tensor.matmul(
                            pk[:], wk_s[:, cc, ec * P : (ec + 1) * P],
                            cbs[cc][:], start=(cc == 0), stop=(cc == 3))
                    for hh in range(2):
                        h4 = 2 * e2 + hh
                        dst = k_res[poff : poff + D, h4, kc_half,
                                    jh * NQ : (jh + 1) * NQ]
                        if hh == 0:
                            nc.scalar.activation(dst, pk[0:D, :], AF.Copy)
                        else:
                            nc.vector.tensor_copy(dst, pk[D : 2 * D, :])
                # v projection, two jblks share one PSUM bank
                for jp in range(2):
                    pv2 = ps.tile([P, NQ], F32, tag="misc", bufs=2)
                    for half in range(2):
                        jblk = 2 * jp + half
                        jc = jh * 4 + jblk
                        for cc in range(4):
                            nc.tensor.matmul(
                                pv2[:, half * 256 : (half + 1) * 256],
                                cbs[cc][:, jblk * P : (jblk + 1) * P],
                                wv_s[:, cc, e0 : e0 + 256],
                                start=(cc == 0), stop=(cc == 3))
                    for half in range(2):
                        jblk = 2 * jp + half
                        jc = jh * 4 + jblk
                        dst = v_res[:, jc, :, r * D : (r + 1) * D]  # [128, 4, 64]
                        src = pv2[:, half * 256 : (half + 1) * 256].rearrange(
                            "p (h d) -> p h d", h=4)
                        if jblk % 2 == 0:
                            nc.scalar.activation(dst, src, AF.Copy)
                        else:
                            nc.vector.tensor_copy(dst, src)

            NCB = CCOLS // NQ
            # ---- interleaved stage A + group-1 kv projection ----
            xf_next = dma_x(0)
            xbs_next = conv_x(xf_next)
            xf_next = dma_x(1)
            for cc in range(4):
                nc.sync.dma_start(wq_s[:, cc, :], wqT[cc * P : (cc + 1) * P, :])
                nc.sync.dma_start(wk_s[:, cc, :], wkT[cc * P : (cc + 1) * P, :])
                nc.sync.dma_start(wv_s[:, cc, :], wvT[cc * P : (cc + 1) * P, :])
            cf_next = dma_ctx(0)
            cbs_next = conv_ctx(cf_next)
            cf_next = dma_ctx(1)
            for i in range(R):
                xbs = xbs_next
                if i + 1 < R:
                    xbs_next = conv_x(xf_next)
                if i + 2 < R:
                    xf_next = dma_x(i + 2)
                a_block(i, xbs)
                for sub in range(2):
                    cb = 2 * i + sub
                    cbs = cbs_next
                    if cb + 1 < NCB:
                        cbs_next = conv_ctx(cf_next)
                    if cb + 2 < NCB:
                        cf_next = dma_ctx(cb + 2)
                    b_block(0, cb, cbs)

            for g in range(2):
                if g == 1:
                    # ---- group-2 kv projection ----
                    cf_next = dma_ctx(0)
                    cbs_next = conv_ctx(cf_next)
                    cf_next = dma_ctx(1)
                    for cb in range(NCB):
                        cbs = cbs_next
                        if cb + 1 < NCB:
                            cbs_next = conv_ctx(cf_next)
                        if cb + 2 < NCB:
                            cf_next = dma_ctx(cb + 2)
                        b_block(1, cb, cbs)
                # ---- attention for the 4 heads of this group ----
                for h4 in range(4):
                    h = 4 * g + h4
                    PT = pt_p.tile([P, JC, NQ], BF16, tag="PT")
                    # null-key logits, broadcast across partitions by the
                    # replicated stationary
                    pn = ps.tile([P, NQ], F32, tag="misc", bufs=2)
                    for kc in range(KC):
                        nc.tensor.matmul(
                            pn[:], nullk_m[:, :], q_res[:, h, kc, :],
                            start=(kc == 0), stop=(kc == KC - 1))
                    en = stat.tile([P, NQ], BF16, tag="en")
                    nc.scalar.activation(en[:], pn[:], AF.Exp, scale=SIM_SCALE)
                    pden = ps.tile([P, NQ], F32, tag="misc", bufs=2)
                    for jb in range(JC):  # 8 key blocks
                        psim = ps.tile([P, NQ], F32, tag="big", bufs=6)
                        for kc in range(KC):
                            nc.tensor.matmul(
                                psim[:], k_res[:, h4, kc, jb * P : (jb + 1) * P],
                                q_res[:, h, kc, :], start=(kc == 0), stop=(kc == KC - 1))
                        nc.scalar.activation(PT[:, jb, :], psim[:], AF.Exp,
                                             scale=SIM_SCALE)
                        # denominator for the PREVIOUS block: its exp is done,
                        # so the PE never waits on the scalar engine here
                        if jb >= 1:
                            nc.tensor.matmul(pden[:], ones_mat[:], PT[:, jb - 1, :],
                                             start=(jb == 1), stop=False)
                    nc.tensor.matmul(pden[:], ones_mat[:], PT[:, JC - 1, :],
                                     start=False, stop=False)
                    # fold the null term in with a K=1 matmul (row 0 of en)
                    nc.tensor.matmul(pden[:], ones_row[:], en[0:1, :],
                                     start=False, stop=True)
                    dln = stat.tile([P, NQ], F32, tag="dln")
                    nc.scalar.activation(dln[:], pden[:], AF.Ln)
                    d_s = stat.tile([P, NQ], F32, tag="d_s")
                    nc.scalar.activation(d_s[:], dln[:], AF.Exp, scale=-1.0)
                    ec, eoff = h // 2, (h % 2) * D
                    # attn @ v, two interleaved PSUM chains per pair so bank
                    # drains overlap; normalized by 1/den on the PSUM copy
                    for pr in range(KC // 2):
                        pavs = []
                        for q2 in range(2):
                            pav = ps.tile([P, NQ], F32, tag="big", bufs=6)
                            pavs.append(pav)
                        for jc in range(JC):
                            for q2 in range(2):
                                rc2 = 2 * pr + q2
                                nc.tensor.matmul(
                                    pavs[q2][:],
                                    v_res[:, jc, h4, rc2 * P : (rc2 + 1) * P],
                                    PT[:, jc, :], start=(jc == 0), stop=False)
                        for q2 in range(2):
                            rc2 = 2 * pr + q2
                            nc.tensor.matmul(pavs[q2][:], nullv_s[:, :], en[0:1, :],
                                             start=False, stop=True)
                            ao = ao_p.tile([P, NQ], BF16, tag="ao")
                            nc.vector.tensor_mul(ao[:], pavs[q2][:], d_s[:])
                            for rr in range(2):
                                rv = 2 * rc2 + rr
                                nc.sync.dma_start(
                                    aoT_d[eoff : eoff + D, ec, rv * NQ : (rv + 1) * NQ],
                                    ao[rr * D : (rr + 1) * D, :])

        # ------------- Stage C: out projection + final LN ---------------
        with tc.tile_pool(name="w3", bufs=1) as w3, \
             tc.tile_pool(name="s3", bufs=4) as s3, \
             tc.tile_pool(name="st3", bufs=6) as st3, \
             tc.tile_pool(name="p3", bufs=8, space="PSUM") as p3:

            wo_s = w3.tile([P, 4, C], BF16)
            for ec in range(4):
                nc.sync.dma_start(wo_s[:, ec, :], woT[ec * P : (ec + 1) * P, :])
            ao_s = w3.tile([P, 4, XCOLS], BF16)
            for oct_ in range(8):
                for ec in range(4):
                    nc.sync.dma_start(
                        ao_s[:, ec, oct_ * 768 : (oct_ + 1) * 768],
                        aoT_d[:, ec, oct_ * 768 : (oct_ + 1) * 768])

            for rc in range(XCOLS // P):  # 48 row chunks
                pf = p3.tile([P, C], F32, tag="pf")
                for ec in range(4):
                    nc.tensor.matmul(
                        pf[:], ao_s[:, ec, rc * P : (rc + 1) * P], wo_s[:, ec, :],
                        start=(ec == 0), stop=(ec == 3))
                # reduce (DVE) and Square+accum (scalar) both run off pf in
                # parallel; var = E[x^2] - mean^2
                nmean = st3.tile([P, 1], F32, tag="nmean")
                nc.vector.reduce_sum(nmean[:], pf[:], axis=AX)
                nc.scalar.mul(nmean[:], nmean[:], -1.0 / C)
                sq3 = s3.tile([P, C], BF16, tag="sq3")
                ssum = st3.tile([P, 1], F32, tag="ssum")
                nc.scalar.activation(sq3[:], pf[:], AF.Square, accum_out=ssum[:])
                msq3 = st3.tile([P, 1], F32, tag="msq3")
                nc.vector.tensor_mul(msq3[:], nmean[:], nmean[:])
                var3 = st3.tile([P, 1], F32, tag="var3")
                nc.scalar.mul(var3[:], ssum[:], 1.0 / C)
                nc.vector.tensor_sub(var3[:], var3[:], msq3[:])
                std3 = st3.tile([P, 1], F32, tag="std3")
                nc.scalar.activation(std3[:], var3[:], AF.Sqrt, bias=eps_P[:])
                inv3 = st3.tile([P, 1], F32, tag="inv3")
                nc.vector.reciprocal(inv3[:], std3[:])
                binv = st3.tile([P, 1], F32, tag="binv")
                nc.vector.tensor_mul(binv[:], nmean[:], inv3[:])
                on = s3.tile([P, C], F32, tag="on")
                nc.scalar.activation(on[:], pf[:], AF.Identity, scale=inv3[:],
                                     bias=binv[:])
                nc.sync.dma_start(out[rc * P : (rc + 1) * P, :], on[:])

    nc.compile()
    return nc


def kernel(x, context, norm_g, to_q_w, to_kv_w, null_kv, to_out_w, out_norm_g):
    import ml_dtypes
    from concourse.bass_utils import run_bass_kernel_spmd

    x = np.asarray(x, dtype=np.float32)
    context = np.asarray(context, dtype=np.float32)
    norm_g = np.asarray(norm_g, dtype=np.float32)
    to_q_w = np.asarray(to_q_w, dtype=np.float32)
    to_kv_w = np.asarray(to_kv_w, dtype=np.float32)
    null_kv = np.asarray(null_kv, dtype=np.float32)
    to_out_w = np.asarray(to_out_w, dtype=np.float32)
    out_norm_g = np.asarray(out_norm_g, dtype=np.float32)

    if "nc" not in _CACHE:
        _CACHE["nc"] = _build_program()
    nc = _CACHE["nc"]

    BF = ml_dtypes.bfloat16
    wq = np.ascontiguousarray((to_q_w * norm_g[None, :]).T)          # [c, e]
    wqs = np.ascontiguousarray(wq.sum(axis=0).reshape(1, E)).astype(BF)
    wq = wq.astype(BF)
    wk = np.ascontiguousarray(to_kv_w[:E].T).astype(BF)
    wv = np.ascontiguousarray(to_kv_w[E:].T).astype(BF)
    wo = np.ascontiguousarray(to_out_w.T).astype(BF)                 # [e, c]
    nullk_a = np.ascontiguousarray(
        np.concatenate([null_kv[0], null_kv[0]]).reshape(P, 1))
    nullv_a = np.ascontiguousarray(
        np.concatenate([null_kv[1], null_kv[1]]).reshape(1, P))

    in_maps = []
    for core in range(8):
        bi, half = core // 2, core % 2
        xs = x[bi, half * NQ : (half + 1) * NQ]          # [512, 12, 512]
        xT_a = np.ascontiguousarray(xs.transpose(2, 1, 0).reshape(C, XCOLS))
        cs = context[bi]                                  # [1024, 12, 512]
        ctxT_a = np.ascontiguousarray(cs.transpose(2, 1, 0).reshape(C, CCOLS))
        in_maps.append(dict(
            xT=xT_a, ctxT=ctxT_a, wqT=wq, wkT=wk, wvT=wv, wqsum=wqs, woT=wo,
            nullk=nullk_a, nullv=nullv_a))

    trace = bool(int(os.environ.get("KERNEL_TRACE", "0")))
    res = run_bass_kernel_spmd(nc, in_maps, list(range(8)), trace=trace)
    _CACHE["last_exec_ns"] = res.exec_time_ns

    outs = []
    for core in range(8):
        o = res.results[core]["out"]                      # [6144, 512], rows (r, i)
        outs.append(o.reshape(R, NQ, C).transpose(1, 0, 2))  # [512, 12, 512]
    full = np.stack(
        [np.concatenate([outs[2 * bi], outs[2 * bi + 1]], axis=0) for bi in range(B)])
    full = full * out_norm_g[None, None, None, :]
    return full.astype(np.float32)


# revision 32
# speedup vs baseline: 1.2011x; 1.2011x over previous
"""Trainium2 Bass kernel for nn_Attention_v2_cross (dense transformer, 8 cores).

Sharding: 8 cores = 4 batches x 2 query-halves. Every core holds the full
weights and full context for its batch (kv projection duplicated across the
pair, zero collectives).

v3 design:
  - Everything SBUF-resident: q (fp8 e4m3) packed [(r,d), i] per head,
    k (fp8) packed [(r,d), j], v (bf16) packed [j, (r,d)]. Only the
    attention output is staged through DRAM (bf16) for the out-projection.
  - ALL matmuls in bf16/fp8 (fp32r lowers to 2-pass fp32_mode=H, 2x slower):
    x/ctx are converted fp32->bf16 on the fly during streaming, split
    between the scalar and vector engines.
  - sim is computed TRANSPOSED: simT[j, i] = k_chunk^T @ q, so exp output
    is directly the attn@v moving operand -- no PE transposes at all.
  - No row-max subtraction (softmax is shift-invariant, logits are O(1)).
  - P = exp(sim) accumulated unnormalized; each head's output scaled by
    1/den on the PSUM->SBUF copy (den = ones-matmul over PT; reciprocal
    broadcast across partitions by a K=1 matmul).
  - LN1 folded into the q projection: q = (W x + (-mean) (x) Wsum) * inv.
  - ctx streamed twice (two 4-head groups) so k/v fit in SBUF.
  - Final LN fused: Square(pf + bias) with accum_out gives the variance
    row-sum in one scalar instruction.
"""

import os
import numpy as np

B, N, R, C = 4, 1024, 12, 512
H, D = 8, 64
E = H * D            # 512
NQ = N // 2          # 512 queries per core
NKJ = N              # 1024 keys per core
ALPHA = 128.0
EPS = 1e-5
XCOLS = R * NQ       # 6144  (col = r*NQ + i)
CCOLS = R * NKJ      # 12288 (col = r*NKJ + j)
P = 128
KC = (R * D) // P    # 6 contraction chunks of 128 over (r,d)
JC = NKJ // P        # 8 key blocks of 128
SIM_SCALE = (D ** -0.5) * (R ** -0.5)   # exp scale; ALPHA cancels in softmax

_CACHE = {}


def _build_program():
    from contextlib import ExitStack
    import concourse.bass as bass
    import concourse.tile as tile
    from concourse import bacc
    from concourse import mybir

    F32 = mybir.dt.float32
    BF16 = mybir.dt.bfloat16
    FP8 = mybir.dt.float8e4
    AF = mybir.ActivationFunctionType
    AX = mybir.AxisListType.X

    nc = bacc.Bacc("TRN2", target_bir_lowering=False, debug=False, num_devices=8)

    xT = nc.dram_tensor("xT", [C, XCOLS], F32, kind="ExternalInput").ap()
    ctxT = nc.dram_tensor("ctxT", [C, CCOLS], F32, kind="ExternalInput").ap()
    wqT = nc.dram_tensor("wqT", [C, E], BF16, kind="ExternalInput").ap()
    wkT = nc.dram_tensor("wkT", [C, E], BF16, kind="ExternalInput").ap()
    wvT = nc.dram_tensor("wvT", [C, E], BF16, kind="ExternalInput").ap()
    wqsum = nc.dram_tensor("wqsum", [1, E], BF16, kind="ExternalInput").ap()
    woT = nc.dram_tensor("woT", [E, C], BF16, kind="ExternalInput").ap()
    nullk = nc.dram_tensor("nullk", [P, 1], F32, kind="ExternalInput").ap()
    nullv = nc.dram_tensor("nullv", [1, P], F32, kind="ExternalInput").ap()
    out = nc.dram_tensor("out", [XCOLS, C], F32, kind="ExternalOutput").ap()

    with ExitStack() as ctx:
        tc = ctx.enter_context(tile.TileContext(nc))

        const = ctx.enter_context(tc.tile_pool(name="const", bufs=1))
        dram = ctx.enter_context(tc.tile_pool(name="dram", bufs=1, space="DRAM"))

        ones_mat = const.tile([P, P], BF16)         # column-sum + broadcast
        nc.vector.memset(ones_mat[:], 1.0)
        ones_row = const.tile([1, P], BF16)         # K=1 partition broadcast
        nc.vector.memset(ones_row[:], 1.0)
        nullk_f = const.tile([P, 1], F32)
        nc.sync.dma_start(nullk_f[:], nullk[:, :])
        nullk_m = const.tile([P, P], FP8)           # null key replicated 128x
        nc.any.tensor_copy(nullk_m[:], nullk_f[:].to_broadcast((P, P)))
        nullv_f = const.tile([1, P], F32)
        nc.sync.dma_start(nullv_f[:], nullv[:, :])
        nullv_s = const.tile([1, P], BF16)
        nc.any.tensor_copy(nullv_s[:], nullv_f[:])
        wqsum_s = const.tile([1, E], BF16)
        nc.sync.dma_start(wqsum_s[:], wqsum[:, :])
        eps_1 = const.tile([1, 1], F32)
        nc.vector.memset(eps_1[:], EPS)
        eps_P = const.tile([P, 1], F32)
        nc.vector.memset(eps_P[:], EPS)

        aoT_d = dram.tile([P, 4, XCOLS], BF16)   # aoT[e, (r,i)]: e = ec*128+p

        with tc.tile_pool(name="res", bufs=1) as res, \
             tc.tile_pool(name="w", bufs=1) as wpool, \
             tc.tile_pool(name="sf", bufs=8) as sf, \
             tc.tile_pool(name="sb", bufs=6) as sb, \
             tc.tile_pool(name="sq", bufs=4) as sq_p, \
             tc.tile_pool(name="stat", bufs=2) as stat, \
             tc.tile_pool(name="pt", bufs=2) as pt_p, \
             tc.tile_pool(name="ao", bufs=3) as ao_p, \
             tc.tile_pool(name="ps", bufs=1, space="PSUM") as ps:

            # Resident activations
            q_res = res.tile([P, H, KC, NQ], FP8)          # [(r%2)*64+d, h, kchunk, i]
            k_res = res.tile([P, 4, KC, NKJ], FP8)         # per group: 4 heads
            v_res = res.tile([P, JC, 4, R * D], BF16)      # [j%128, jc, h, (r,d)]

            wq_s = wpool.tile([P, 4, E], BF16)
            wk_s = wpool.tile([P, 4, E], BF16)
            wv_s = wpool.tile([P, 4, E], BF16)

            # ------------- Stage A: LN(x) folded into q projection -------
            def dma_x(rb):
                tiles = []
                for cc in range(4):
                    t = sf.tile([P, NQ], F32, tag="xf", bufs=6)
                    nc.sync.dma_start(t[:], xT[cc * P : (cc + 1) * P, rb * NQ : (rb + 1) * NQ])
                    tiles.append(t)
                return tiles

            def conv_x(fts):
                tiles = []
                for cc in range(4):
                    xb = sb.tile([P, NQ], BF16, tag="xb", bufs=8)
                    if cc % 2 == 0:
                        nc.scalar.activation(xb[:], fts[cc][:], AF.Copy)
                    else:
                        nc.vector.tensor_copy(xb[:], fts[cc][:])
                    tiles.append(xb)
                return tiles

            # two-deep pipeline: DMA two blocks ahead, convert one ahead, so
            # the casts at the queue heads never wait on an in-flight DMA
            xf_next = dma_x(0)
            for cc in range(4):
                nc.sync.dma_start(wq_s[:, cc, :], wqT[cc * P : (cc + 1) * P, :])
            xbs_next = conv_x(xf_next)
            xf_next = dma_x(1)
            for rb in range(R):
                xbs = xbs_next
                if rb + 1 < R:
                    xbs_next = conv_x(xf_next)
                if rb + 2 < R:
                    xf_next = dma_x(rb + 2)
                psum_sum = ps.tile([P, NQ], F32, tag="misc", bufs=2)
                for cc in range(4):
                    nc.tensor.matmul(psum_sum[:], ones_mat[:], xbs[cc][:],
                                     start=(cc == 0), stop=(cc == 3))
                # negm first: frees the stats PSUM buffer early
                negm_b = stat.tile([P, NQ], BF16, tag="negm")
                nc.scalar.mul(negm_b[:], psum_sum[:], -1.0 / C)
                kc_half, poff = rb // 2, (rb % 2) * D
                pqs = []
                for ec in range(4):
                    pq = ps.tile([P, NQ], F32, tag="big", bufs=6)
                    for cc in range(4):
                        nc.tensor.matmul(
                            pq[:], wq_s[:, cc, ec * P : (ec + 1) * P],
                            xbs[cc][:], start=(cc == 0), stop=False)
                    pqs.append(pq)
                sqs = []
                for cc in range(4):
                    sq = sq_p.tile([P, NQ], BF16, tag="sq")
                    nc.scalar.activation(sq[:], xbs[cc][:], AF.Square)
                    sqs.append(sq)
                psum_sq = ps.tile([P, NQ], F32, tag="misc", bufs=2)
                for cc in range(4):
                    nc.tensor.matmul(psum_sq[:], ones_mat[:], sqs[cc][:],
                                     start=(cc == 0), stop=(cc == 3))
                for ec in range(4):
                    nc.tensor.matmul(
                        pqs[ec][:], wqsum_s[:, ec * P : (ec + 1) * P],
                        negm_b[0:1, :], start=False, stop=True)
                msq = stat.tile([P, NQ], F32, tag="msq")
                nc.vector.tensor_mul(msq[:], negm_b[:], negm_b[:])
                var = stat.tile([P, NQ], F32, tag="var")
                nc.vector.scalar_tensor_tensor(
                    var[:], psum_sq[:], 1.0 / C, msq[:],
                    mybir.AluOpType.mult, mybir.AluOpType.subtract)
                lnv = stat.tile([P, NQ], F32, tag="var")
                nc.scalar.activation(lnv[:], var[:], AF.Ln, bias=eps_P[:])
                a_s = stat.tile([P, NQ], F32, tag="a_s")
                nc.scalar.activation(a_s[:], lnv[:], AF.Exp, scale=-0.5)
                for ec in range(4):
                    for hh in range(2):
                        h = 2 * ec + hh
                        dst = q_res[poff : poff + D, h, kc_half, :]
                        src = pqs[ec][hh * D : (hh + 1) * D, :]
                        av = a_s[hh * D : (hh + 1) * D, :]
                        nc.vector.tensor_mul(dst, src, av)

            # ------------- Per-group: kv projection + attention ----------
            for cc in range(4):
                nc.sync.dma_start(wk_s[:, cc, :], wkT[cc * P : (cc + 1) * P, :])
                nc.sync.dma_start(wv_s[:, cc, :], wvT[cc * P : (cc + 1) * P, :])

            def dma_ctx(cb):
                tiles = []
                for cc in range(4):
                    t = sf.tile([P, NQ], F32, tag="cf")
                    nc.sync.dma_start(
                        t[:], ctxT[cc * P : (cc + 1) * P, cb * NQ : (cb + 1) * NQ])
                    tiles.append(t)
                return tiles

            def conv_ctx(fts):
                tiles = []
                for cc in range(4):
                    cbt = sb.tile([P, NQ], BF16, tag="cb", bufs=10)
                    if cc % 2 == 0:
                        nc.scalar.activation(cbt[:], fts[cc][:], AF.Copy)
                    else:
                        nc.vector.tensor_copy(cbt[:], fts[cc][:])
                    tiles.append(cbt)
                return tiles

            NCB = CCOLS // NQ
            for g in range(2):
                e0 = g * 256  # e-offset of this 4-head group
                cf_next = dma_ctx(0)
                cbs_next = conv_ctx(cf_next)
                cf_next = dma_ctx(1)
                for cb in range(NCB):  # 24 blocks: (r, j-half)
                    r, jh = cb // 2, cb % 2
                    kc_half, poff = r // 2, (r % 2) * D
                    cbs = cbs_next
                    if cb + 1 < NCB:
                        cbs_next = conv_ctx(cf_next)
                    if cb + 2 < NCB:
                        cf_next = dma_ctx(cb + 2)
                    # k projection (2 e-chunks = 4 heads), transposed out [e, j]
                    for e2 in range(2):
                        ec = 2 * g + e2
                        pk = ps.tile([P, NQ], F32, tag="big", bufs=6)
                        for cc in range(4):
                            nc.tensor.matmul(
                                pk[:], wk_s[:, cc, ec * P : (ec + 1) * P],
                                cbs[cc][:], start=(cc == 0), stop=(cc == 3))
                        for hh in range(2):
                            h4 = 2 * e2 + hh  # head index within group
                            dst = k_res[poff : poff + D, h4, kc_half,
                                        jh * NQ : (jh + 1) * NQ]
                            if hh == 0:
                                nc.scalar.activation(dst, pk[0:D, :], AF.Copy)
                            else:
                                nc.vector.tensor_copy(dst, pk[D : 2 * D, :])
                    # v projection, row-major [j, e-group]; batched copy
                    for jblk in range(4):
                        jc = jh * 4 + jblk
                        pv = ps.tile([P, NQ], F32, tag="misc", bufs=2)
                        for cc in range(4):
                            nc.tensor.matmul(
                                pv[:, 0:256], cbs[cc][:, jblk * P : (jblk + 1) * P],
                                wv_s[:, cc, e0 : e0 + 256],
                                start=(cc == 0), stop=(cc == 3))
                        dst = v_res[:, jc, :, r * D : (r + 1) * D]  # [128, 4, 64]
                        src = pv[:, 0:256].rearrange("p (h d) -> p h d", h=4)
                        if jblk % 2 == 0:
                            nc.scalar.activation(dst, src, AF.Copy)
                        else:
                            nc.vector.tensor_copy(dst, src)

                # ---- attention for the 4 heads of this group ----
                for h4 in range(4):
                    h = 4 * g + h4
                    PT = pt_p.tile([P, JC, NQ], BF16, tag="PT")
                    # null-key logits, broadcast across partitions by the
                    # replicated stationary
                    pn = ps.tile([P, NQ], F32, tag="misc", bufs=2)
                    for kc in range(KC):
                        nc.tensor.matmul(
                            pn[:], nullk_m[:, :], q_res[:, h, kc, :],
                            start=(kc == 0), stop=(kc == KC - 1))
                    en = stat.tile([P, NQ], BF16, tag="en")
                    nc.scalar.activation(en[:], pn[:], AF.Exp, scale=SIM_SCALE)
                    pden = ps.tile([P, NQ], F32, tag="misc", bufs=2)
                    for jb in range(JC):  # 8 key blocks
                        psim = ps.tile([P, NQ], F32, tag="big", bufs=6)
                        for kc in range(KC):
                            nc.tensor.matmul(
                                psim[:], k_res[:, h4, kc, jb * P : (jb + 1) * P],
                                q_res[:, h, kc, :], start=(kc == 0), stop=(kc == KC - 1))
                        nc.scalar.activation(PT[:, jb, :], psim[:], AF.Exp,
                                             scale=SIM_SCALE)
                        # denominator for the PREVIOUS block: its exp is done,
                        # so the PE never waits on the scalar engine here
                        if jb >= 1:
                            nc.tensor.matmul(pden[:], ones_mat[:], PT[:, jb - 1, :],
                                             start=(jb == 1), stop=False)
                    nc.tensor.matmul(pden[:], ones_mat[:], PT[:, JC - 1, :],
                                     start=False, stop=False)
                    # fold the null term in with a K=1 matmul (row 0 of en)
                    nc.tensor.matmul(pden[:], ones_row[:], en[0:1, :],
                                     start=False, stop=True)
                    dln = stat.tile([P, NQ], F32, tag="dln")
                    nc.scalar.activation(dln[:], pden[:], AF.Ln)
                    d_s = stat.tile([P, NQ], F32, tag="d_s")
                    nc.scalar.activation(d_s[:], dln[:], AF.Exp, scale=-1.0)
                    ec, eoff = h // 2, (h % 2) * D
                    # attn @ v, two interleaved PSUM chains per pair so bank
                    # drains overlap; normalized by 1/den on the PSUM copy
                    for pr in range(KC // 2):
                        pavs = []
                        for q2 in range(2):
                            pav = ps.tile([P, NQ], F32, tag="big", bufs=6)
                            pavs.append(pav)
                        for jc in range(JC):
                            for q2 in range(2):
                                rc2 = 2 * pr + q2
                                nc.tensor.matmul(
                                    pavs[q2][:],
                                    v_res[:, jc, h4, rc2 * P : (rc2 + 1) * P],
                                    PT[:, jc, :], start=(jc == 0), stop=False)
                        for q2 in range(2):
                            rc2 = 2 * pr + q2
                            nc.tensor.matmul(pavs[q2][:], nullv_s[:, :], en[0:1, :],
                                             start=False, stop=True)
                            ao = ao_p.tile([P, NQ], BF16, tag="ao")
                            nc.vector.tensor_mul(ao[:], pavs[q2][:], d_s[:])
                            for rr in range(2):
                                rv = 2 * rc2 + rr
                                nc.sync.dma_start(
                                    aoT_d[eoff : eoff + D, ec, rv * NQ : (rv + 1) * NQ],
                                    ao[rr * D : (rr + 1) * D, :])

        # ------------- Stage C: out projection + final LN ---------------
        with tc.tile_pool(name="w3", bufs=1) as w3, \
             tc.tile_pool(name="s3", bufs=4) as s3, \
             tc.tile_pool(name="st3", bufs=6) as st3, \
             tc.tile_pool(name="p3", bufs=8, space="PSUM") as p3:

            wo_s = w3.tile([P, 4, C], BF16)
            for ec in range(4):
                nc.sync.dma_start(wo_s[:, ec, :], woT[ec * P : (ec + 1) * P, :])
            ao_s = w3.tile([P, 4, XCOLS], BF16)
            for oct_ in range(8):
                for ec in range(4):
                    nc.sync.dma_start(
                        ao_s[:, ec, oct_ * 768 : (oct_ + 1) * 768],
                        aoT_d[:, ec, oct_ * 768 : (oct_ + 1) * 768])

            for rc in range(XCOLS // P):  # 48 row chunks
                pf = p3.tile([P, C], F32, tag="pf")
                for ec in range(4):
                    nc.tensor.matmul(
                        pf[:], ao_s[:, ec, rc * P : (rc + 1) * P], wo_s[:, ec, :],
                        start=(ec == 0), stop=(ec == 3))
                # reduce (DVE) and Square+accum (scalar) both run off pf in
                # parallel; var = E[x^2] - mean^2
                nmean = st3.tile([P, 1], F32, tag="nmean")
                nc.vector.reduce_sum(nmean[:], pf[:], axis=AX)
                nc.scalar.mul(nmean[:], nmean[:], -1.0 / C)
                sq3 = s3.tile([P, C], BF16, tag="sq3")
                ssum = st3.tile([P, 1], F32, tag="ssum")
                nc.scalar.activation(sq3[:], pf[:], AF.Square, accum_out=ssum[:])
                msq3 = st3.tile([P, 1], F32, tag="msq3")
                nc.vector.tensor_mul(msq3[:], nmean[:], nmean[:])
                var3 = st3.tile([P, 1], F32, tag="var3")
                nc.scalar.mul(var3[:], ssum[:], 1.0 / C)
                nc.vector.tensor_sub(var3[:], var3[:], msq3[:])
                std3 = st3.tile([P, 1], F32, tag="std3")
                nc.scalar.activation(std3[:], var3[:], AF.Sqrt, bias=eps_P[:])
                inv3 = st3.tile([P, 1], F32, tag="inv3")
                nc.vector.reciprocal(inv3[:], std3[:])
                binv = st3.tile([P, 1], F32, tag="binv")
                nc.vector.tensor_mul(binv[:], nmean[:], inv3[:])
                on = s3.tile([P, C], F32, tag="on")
                nc.scalar.activation(on[:], pf[:], AF.Identity, scale=inv3[:],
                                     bias=binv[:])
                nc.sync.dma_start(out[rc * P : (rc + 1) * P, :], on[:])

    nc.compile()
    return nc


def kernel(x, context, norm_g, to_q_w, to_kv_w, null_kv, to_out_w, out_norm_g):
    import ml_dtypes
    from concourse.bass_utils import run_bass_kernel_spmd

    x = np.asarray(x, dtype=np.float32)
    context = np.asarray(context, dtype=np.float32)
    norm_g = np.asarray(norm_g, dtype=np.float32)
    to_q_w = np.asarray(to_q_w, dtype=np.float32)
    to_kv_w = np.asarray(to_kv_w, dtype=np.float32)
    null_kv = np.asarray(null_kv, dtype=np.float32)
    to_out_w = np.asarray(to_out_w, dtype=np.float32)
    out_norm_g = np.asarray(out_norm_g, dtype=np.float32)

    if "nc" not in _CACHE:
        _CACHE["nc"] = _build_program()
    nc = _CACHE["nc"]

    BF = ml_dtypes.bfloat16
    wq = np.ascontiguousarray((to_q_w * norm_g[None, :]).T)          # [c, e]
    wqs = np.ascontiguousarray(wq.sum(axis=0).reshape(1, E)).astype(BF)
    wq = wq.astype(BF)
    wk = np.ascontiguousarray(to_kv_w[:E].T).astype(BF)
    wv = np.ascontiguousarray(to_kv_w[E:].T).astype(BF)
    wo = np.ascontiguousarray(to_out_w.T).astype(BF)                 # [e, c]
    nullk_a = np.ascontiguousarray(
        np.concatenate([null_kv[0], null_kv[0]]).reshape(P, 1))
    nullv_a = np.ascontiguousarray(
        np.concatenate([null_kv[1], null_kv[1]]).reshape(1, P))

    in_maps = []
    for core in range(8):
        bi, half = core // 2, core % 2
        xs = x[bi, half * NQ : (half + 1) * NQ]          # [512, 12, 512]
        xT_a = np.ascontiguousarray(xs.transpose(2, 1, 0).reshape(C, XCOLS))
        cs = context[bi]                                  # [1024, 12, 512]
        ctxT_a = np.ascontiguousarray(cs.transpose(2, 1, 0).reshape(C, CCOLS))
        in_maps.append(dict(
            xT=xT_a, ctxT=ctxT_a, wqT=wq, wkT=wk, wvT=wv, wqsum=wqs, woT=wo,
            nullk=nullk_a, nullv=nullv_a))

    trace = bool(int(os.environ.get("KERNEL_TRACE", "0")))
    res = run_bass_kernel_spmd(nc, in_maps, list(range(8)), trace=trace)
    _CACHE["last_exec_ns"] = res.exec_time_ns

    outs = []
    for core in range(8):
        o = res.results[core]["out"]                      # [6144, 512], rows (r, i)
        outs.append(o.reshape(R, NQ, C).transpose(1, 0, 2))  # [512, 12, 512]
    full = np.stack(
        [np.concatenate([outs[2 * bi], outs[2 * bi + 1]], axis=0) for bi in range(B)])
    full = full * out_norm_g[None, None, None, :]
    return full.astype(np.float32)
